# revision 1
# baseline (speedup 1.0000x reference)
"""Multi-head attention (B=4, S=2048, D=1024, 16 heads) on 8 TRN2 NeuronCores.

Sharding: core c = (batch b = c//2, head-group g = c%2). Each core computes the
attention output for its batch over its 8 heads plus the out-projection partial
for those heads' feature columns; the host sums the two per-batch partials.

Per-core Bass/Tile kernel (all-bf16 compute, fp32 PSUM accumulation):
  QT = (0.125*Wq_g).T-applied, feature-major [512, 2048]; KT likewise
  V seq-major [2048, 8*(64+1)] with a ones column per head (gives the softmax
  denominator for free during the AV matmul)
  per head: ST[k,q] (K=64 matmuls) -> exp on ACT (no max subtraction; scores
  are O(1) by construction) -> P[k,q] bf16 -> AV: y[q, 65] = P.T @ [V|1],
  normalized by the ones-column sum via per-partition tensor_scalar
  y -> PE-transpose -> yT -> out = yT.T @ Wo_g.T partial [2048, 1024] fp32

Schedule: 16 software-pipelined iterations (q-chunk x head); scores of
iteration i interleave with the AV of iteration i-1 at key-tile granularity;
QKV projections, transposes, out-projection and keep-warm filler matmuls are
spread into the ACT-paced slack so the PE never idles long enough for the HAM
clock gate to re-throttle.
"""

from contextlib import ExitStack

import ml_dtypes
import numpy as np

import concourse.bass as bass
import concourse.tile as tile
from concourse import bacc, mybir
from concourse.bass_utils import run_bass_kernel_spmd

F32 = mybir.dt.float32
BF16 = mybir.dt.bfloat16
BF = ml_dtypes.bfloat16

B = 4
S = 2048
D = 1024
NH = 16
HD = 64
G = 2  # head groups (tensor-parallel)
FH = D // G  # 512 local features
NHL = NH // G  # 8 local heads
KT_D = D // 128  # 8
ST_S = S // 128  # 16
FT = FH // 128  # 4
QC = 1024  # attention q-chunk
NQC = S // QC  # 2
NKT = S // 128  # 16
NIT = NQC * NHL  # 16 pipelined attention iterations
N_CORES = 8


def _build_nc():
    nc = bacc.Bacc("TRN2", debug=False, num_devices=N_CORES, target_bir_lowering=False)

    xt_d = nc.dram_tensor("xt", [D, S], BF16, kind="ExternalInput").ap()
    wq_d = nc.dram_tensor("wq", [D, FH], BF16, kind="ExternalInput").ap()
    wk_d = nc.dram_tensor("wk", [D, FH], BF16, kind="ExternalInput").ap()
    wv_d = nc.dram_tensor("wv", [D, FH], BF16, kind="ExternalInput").ap()
    wo_d = nc.dram_tensor("wo", [FH, D], BF16, kind="ExternalInput").ap()
    id_d = nc.dram_tensor("ident", [128, 128], BF16, kind="ExternalInput").ap()
    out_d = nc.dram_tensor("out", [S, D], F32, kind="ExternalOutput").ap()

    with tile.TileContext(nc) as tc, ExitStack() as ctx:
        pool_const = ctx.enter_context(tc.tile_pool(name="const", bufs=1))
        pool_xt = ctx.enter_context(tc.tile_pool(name="xt", bufs=1))
        pool_w = ctx.enter_context(tc.tile_pool(name="w", bufs=1))
        pool_qk = ctx.enter_context(tc.tile_pool(name="qk", bufs=1))
        pool_v = ctx.enter_context(tc.tile_pool(name="v", bufs=1))
        pool_p = ctx.enter_context(tc.tile_pool(name="p", bufs=17))
        pool_y = ctx.enter_context(tc.tile_pool(name="y", bufs=1))
        pool_yt = ctx.enter_context(tc.tile_pool(name="yt", bufs=1))
        pool_sm = ctx.enter_context(tc.tile_pool(name="sm", bufs=4))
        pool_ob = ctx.enter_context(tc.tile_pool(name="ob", bufs=3))
        pool_st = ctx.enter_context(tc.tile_pool(name="st", bufs=2, space="PSUM"))
        pool_yp = ctx.enter_context(tc.tile_pool(name="yp", bufs=1, space="PSUM"))
        pool_ex = ctx.enter_context(tc.tile_pool(name="ex", bufs=2, space="PSUM"))

        ident = pool_const.tile([128, 128], BF16, name="ident")
        nc.sync.dma_start(ident[:], id_d[:])

        xt_sb = []
        for k in range(KT_D):
            t = pool_xt.tile([128, S], BF16, name=f"xt{k}")
            nc.sync.dma_start(t[:], xt_d[bass.ts(k, 128), :])
            xt_sb.append(t)

        w_sb = {}
        for wname, wd in (("wq", wq_d), ("wk", wk_d), ("wv", wv_d)):
            tiles = []
            for k in range(KT_D):
                t = pool_w.tile([128, FH], BF16, name=f"{wname}{k}")
                nc.sync.dma_start(t[:], wd[bass.ts(k, 128), :])
                tiles.append(t)
            w_sb[wname] = tiles
        wo_sb = []
        for f in range(FT):
            t = pool_w.tile([128, D], BF16, name=f"wo{f}")
            nc.sync.dma_start(t[:], wo_d[bass.ts(f, 128), :])
            wo_sb.append(t)

        qt_sb = [pool_qk.tile([128, S], BF16, name=f"qt{t}") for t in range(FT)]
        kt_sb = [pool_qk.tile([128, S], BF16, name=f"kt{t}") for t in range(FT)]
        v_sb = [
            pool_v.tile([128, NHL * (HD + 1)], BF16, name=f"v{s}") for s in range(ST_S)
        ]
        y_sb = [pool_y.tile([128, FH], BF16, name=f"y{q}") for q in range(ST_S)]
        yt_sb = [pool_yt.tile([128, S], BF16, name=f"yt{f}") for f in range(FT)]

        def gen_qk_chain(wname, dst, f, cc):
            ps = pool_ex.tile([128, 512], F32, name="ex")
            for k in range(KT_D):
                nc.tensor.matmul(
                    ps[:],
                    w_sb[wname][k][:, bass.ts(f, 128)],
                    xt_sb[k][:, bass.ts(cc, 512)],
                    start=(k == 0),
                    stop=(k == KT_D - 1),
                )
            nc.vector.tensor_copy(dst[f][:, bass.ts(cc, 512)], ps[:])

        def gen_v_chain(s):
            ps = pool_ex.tile([128, FH], F32, name="ex")
            for k in range(KT_D):
                nc.tensor.matmul(
                    ps[:],
                    xt_sb[k][:, bass.ts(s, 128)],
                    w_sb["wv"][k][:],
                    start=(k == 0),
                    stop=(k == KT_D - 1),
                )
            v3 = v_sb[s].rearrange("p (h c) -> p h c", c=HD + 1)
            nc.vector.tensor_copy(
                v3[:, :, 0:HD], ps.rearrange("p (h d) -> p h d", d=HD)
            )
            nc.vector.memset(v3[:, :, HD : HD + 1], 1.0)

        def gen_transpose(qi, f):
            tp = pool_ex.tile([128, 128], BF16, name="ex")
            nc.tensor.transpose(tp[:], y_sb[qi][:, bass.ts(f, 128)], ident[:])
            nc.vector.tensor_copy(yt_sb[f][:, bass.ts(qi, 128)], tp[:])

        def gen_outproj(qi, e):
            ps = pool_ex.tile([128, 512], F32, name="ex")
            for f in range(FT):
                nc.tensor.matmul(
                    ps[:],
                    yt_sb[f][:, bass.ts(qi, 128)],
                    wo_sb[f][:, bass.ts(e, 512)],
                    start=(f == 0),
                    stop=(f == FT - 1),
                )
            ob = pool_ob.tile([128, 512], F32, name="ob")
            nc.vector.tensor_copy(ob[:], ps[:])
            nc.sync.dma_start(out_d[bass.ts(qi, 128), bass.ts(e, 512)], ob[:])

        def tailwork_units(qi):
            units = [(lambda qi=qi, f=f: gen_transpose(qi, f)) for f in range(FT)]
            units += [(lambda qi=qi, e=e: gen_outproj(qi, e)) for e in range(2)]
            return units

        def gen_dummy():
            # keep-warm matmul: dense 512-col stream into a scratch psum bank
            # that is never read; prevents HAM from re-throttling the PE
            # during ACT-paced stretches with no useful PE slack-work left.
            ps = pool_ex.tile([128, 512], F32, name="ex")
            nc.tensor.matmul(ps[:], ident[:], kt_sb[0][:, 0:512], start=True, stop=True)

        extras = {it: [] for it in range(NIT)}
        for s in range(ST_S):  # V proj inside it 0
            extras[0].append(lambda s=s: gen_v_chain(s))
        qk_slots = {1: [1], 2: [2], 3: [3, 4]}
        for f in range(1, FT):
            its = qk_slots[f]
            for i, (wname, dst) in enumerate((("wq", qt_sb), ("wk", kt_sb))):
                for cc in range(S // 512):
                    unit_idx = i * 4 + cc
                    target_it = its[unit_idx * len(its) // 8]
                    extras[target_it].append(
                        lambda wname=wname, dst=dst, f=f, cc=cc: gen_qk_chain(
                            wname, dst, f, cc
                        )
                    )
        for it in range(5, 9):  # keep-warm fillers in the bare iterations
            extras[it].extend([gen_dummy] * 12)
        for it in range(9, NIT):  # tailwork for chunk-0 q-tiles in its 9..15
            extras[it].extend(tailwork_units(it - 9))
            extras[it].extend([gen_dummy] * 6)

        def yp_off(j):
            return j * (HD + 1) if j < 4 else 512 + (j - 4) * (HD + 1)

        def av_kt(yp, c, h, p_tiles, kt):
            # start=True clears has_written for the WHOLE bank, so only the
            # first matmul touching each bank (j=0 -> bank A, j=4 -> bank B)
            # may set it; the other j groups overwrite-on-clear-bit at kt=0.
            for j in range(QC // 128):
                nc.tensor.matmul(
                    yp[:, yp_off(j) : yp_off(j) + HD + 1],
                    p_tiles[kt][:, bass.ts(j, 128)],
                    v_sb[kt][:, h * (HD + 1) : (h + 1) * (HD + 1)],
                    start=(kt == 0 and j in (0, 4)),
                    stop=(kt == NKT - 1),
                    skip_group_check=True,
                )

        def normalize(yp, c, h):
            y2 = yp.rearrange("p (b r) -> p b r", b=2)  # [128, 2, 512]
            yj = y2[:, :, 0 : 4 * (HD + 1)].rearrange(
                "p b (j c) -> p b j c", c=HD + 1
            )  # [128, 2, 4, 65]
            rcp = pool_sm.tile([128, 8], F32, name="rcp")
            nc.vector.reciprocal(
                rcp.rearrange("p (b j c) -> p b j c", b=2, c=1),
                yj[:, :, :, HD : HD + 1],
            )
            for j in range(QC // 128):
                qi = c * (QC // 128) + j
                nc.vector.tensor_scalar_mul(
                    y_sb[qi][:, h * HD : (h + 1) * HD],
                    yp[:, yp_off(j) : yp_off(j) + HD],
                    rcp[:, j : j + 1],
                )

        # ---- main schedule ----
        for wname, dst in (("wq", qt_sb), ("wk", kt_sb)):
            for cc in range(S // 512):
                gen_qk_chain(wname, dst, 0, cc)

        prev = None  # (yp, c, h, p_tiles)
        for it in range(NIT):
            c, h = divmod(it, NHL)
            ft, ro = h // 2, (h % 2) * HD
            ex_units = list(extras[it])
            n_ex = len(ex_units)
            p_tiles = []
            for kt in range(NKT):
                st = pool_st.tile([128, QC], F32, name="st")
                for half in range(QC // 512):
                    q0 = c * QC + half * 512
                    nc.tensor.matmul(
                        st[:, bass.ts(half, 512)],
                        kt_sb[ft][ro : ro + HD, bass.ts(kt, 128)],
                        qt_sb[ft][ro : ro + HD, q0 : q0 + 512],
                        start=True,
                        stop=True,
                    )
                pt = pool_p.tile([128, QC], BF16, name="p")
                nc.scalar.activation(pt[:], st[:], mybir.ActivationFunctionType.Exp)
                p_tiles.append(pt)
                if prev is not None:
                    pyp, pc, ph, pp = prev
                    av_kt(pyp, pc, ph, pp, kt)
                lo = (kt * n_ex) // NKT
                hi = ((kt + 1) * n_ex) // NKT
                for u in ex_units[lo:hi]:
                    u()
            if prev is not None:
                normalize(prev[0], prev[1], prev[2])
            yp = pool_yp.tile([128, 1024], F32, name="yp")
            prev = (yp, c, h, p_tiles)

        # drain: AV + normalize for the last iteration, then remaining tailwork
        yp, c, h, p_tiles = prev
        for kt in range(NKT):
            av_kt(yp, c, h, p_tiles, kt)
        normalize(yp, c, h)
        for qi in range(7, ST_S):
            for u in tailwork_units(qi):
                u()

    nc.compile()
    return nc


_NC_CACHE = []


def _get_nc():
    if not _NC_CACHE:
        _NC_CACHE.append(_build_nc())
    return _NC_CACHE[0]


def make_in_maps(x, Wq, Wk, Wv, Wo):
    ident = np.eye(128, dtype=BF)
    scale = np.float32(1.0 / np.sqrt(HD))
    in_maps = []
    for c in range(N_CORES):
        b, g = divmod(c, G)
        rows = slice(g * FH, (g + 1) * FH)
        in_maps.append(
            {
                "xt": np.ascontiguousarray(x[b].T).astype(BF),
                "wq": np.ascontiguousarray((Wq[rows, :] * scale).T).astype(BF),
                "wk": np.ascontiguousarray(Wk[rows, :].T).astype(BF),
                "wv": np.ascontiguousarray(Wv[rows, :].T).astype(BF),
                "wo": np.ascontiguousarray(Wo[:, rows].T).astype(BF),
                "ident": ident,
            }
        )
    return in_maps


def kernel(x, Wq, Wk, Wv, Wo):
    x = np.asarray(x, dtype=np.float32)
    Wq = np.asarray(Wq, dtype=np.float32)
    Wk = np.asarray(Wk, dtype=np.float32)
    Wv = np.asarray(Wv, dtype=np.float32)
    Wo = np.asarray(Wo, dtype=np.float32)

    nc = _get_nc()
    in_maps = make_in_maps(x, Wq, Wk, Wv, Wo)
    res = run_bass_kernel_spmd(nc, in_maps, core_ids=list(range(N_CORES)))

    out = np.zeros((B, S, D), dtype=np.float32)
    for c in range(N_CORES):
        out[c // G] += res.results[c]["out"]
    return out
